# revision 26
# baseline (speedup 1.0000x reference)
"""Trainium2 Bass kernel: differentiable-optics PSF (batch=128, 2 focus, 3 ch).

Math per image (b, f, i): pupil P = Q ∘ (g g^T), Q = A*exp(i*2pi*O_f/lam_i)
(shared per (f,i)), g = exp(i*a*w(v)) per-image separable defocus.
Sampled field taps: field = Fc P Fc^T = (Fc diag(g)) Q (diag(g) Fc^T)
                          = X^T Q^T ... with X = diag(g) Fc^T  [256, 64] cplx.
Stage A:  V = Q X   (lhsT = Q^T tiles, rhs = X)      -> PSUM [256, 64] cplx
Stage B:  per image one PSUM tile [128,128] of all 4 products
          [Vr|Vi]^T [Xr|Xi] ; fieldR^T = P11 - P22', fieldI^T = P12 + P21.
Images grouped 8 per (f, i) so stage A batches N=512 and post-field
element-wise work runs on [64, 512] tiles.
psf^T -> y-blend (2-tap, elementwise) -> Wx matmul -> [32, (img,32)],
batched normalize + 32x32 block transpose at the end (as v1).
"""
import numpy as np
from math import factorial

GRID = 256
FOV = 32
NZ = 15
F_MM = 25.0
F_NUMBER = 2.0
PIXEL_SIZE = 3.45e-6
F_M = F_MM * 1e-3
PUPIL_DIAM = F_M / F_NUMBER
BATCH = 128
NCORES = 8
BPC = BATCH // NCORES          # batch per core (16)
NIMG = BPC * 2 * 3             # images per core (96)
G = 8                          # images per group
NGRP = NIMG // G               # 12 groups per core


def _noll_to_nm(j):
    n = 0
    k = j - 1
    while k > n:
        n += 1
        k -= n
    m = (-1) ** j * ((n % 2) + 2 * ((k + ((n + 1) % 2)) // 2))
    return n, m


def _zernike(n, m, r, theta):
    am = abs(m)
    R = np.zeros_like(r)
    for s in range((n - am) // 2 + 1):
        c = ((-1) ** s * factorial(n - s)
             / (factorial(s) * factorial((n + am) // 2 - s)
                * factorial((n - am) // 2 - s)))
        R = R + c * r ** (n - 2 * s)
    norm = np.sqrt(n + 1) if m == 0 else np.sqrt(2 * (n + 1))
    ang = np.cos(am * theta) if m >= 0 else np.sin(am * theta)
    return np.where(r <= 1.0, norm * R * ang, 0.0)


def _host_consts(wavelengths):
    """Input-independent structural constants (DFT/sampling matrices)."""
    N = GRID
    # shifted DFT: field = Ft X Ft, Ft[a,b] = F[(a+128)%256,(b+128)%256]
    idx = (np.arange(N) + N // 2) % N
    jk = np.outer(idx, idx).astype(np.float64)
    ang = -2.0 * np.pi * jk / N
    Fr_full = np.cos(ang)
    Fi_full = np.sin(ang)

    csel = np.zeros((3, 64), np.int64)      # tap rows per channel
    wt0 = np.zeros((3, 32), np.float32)     # tap weights
    wt1 = np.zeros((3, 32), np.float32)
    for i in range(3):
        lam = float(wavelengths[i])
        zoom = PIXEL_SIZE * FOV * PUPIL_DIAM / (lam * F_M * GRID)
        g1 = (np.arange(FOV, dtype=np.float32) / np.float32(FOV - 1)
              * np.float32(2.0 * zoom) - np.float32(zoom))
        x = ((g1 + 1.0) * GRID - 1.0) * 0.5
        x0 = np.floor(x)
        tx = (x - x0).astype(np.float32)
        x0 = x0.astype(np.int64)
        csel[i, 0:32] = x0
        csel[i, 32:64] = x0 + 1
        wt0[i] = 1.0 - tx
        wt1[i] = tx
    return Fr_full, Fi_full, csel, wt0, wt1


def build_nc():
    import concourse.bass as bass
    import concourse.bacc as bacc
    import concourse.mybir as mybir
    from concourse.tile import TileContext

    f32 = mybir.dt.float32
    bf16 = mybir.dt.bfloat16
    AF = mybir.ActivationFunctionType
    OP = mybir.AluOpType
    TWO_PI = float(2.0 * np.pi)

    nc = bacc.Bacc("TRN2", target_bir_lowering=False)
    # device inputs (per core)
    # Q^T tiles: per (f,i) p6: [QrT(ktmt 4) | QiT(4) | QinT(4)] each [128,128]
    qtd = nc.declare_dram_parameter("qt", [128, 6 * 12 * 128], bf16,
                                    isOutput=False)
    # X-build consts per channel i: [A_t0 | flip(A_t1)] with A = [FcrT|FciT],
    # B = [-FciT|FcrT].  Tile-1 halves are partition-reversed so tile-0 g
    # scalars apply (w(v) is symmetric); Q^T tiles are flipped to match.
    abd = nc.declare_dram_parameter("ab", [128, 3 * 256], bf16,
                                    isOutput=False)
    bbd = nc.declare_dram_parameter("bb", [128, 3 * 256], bf16,
                                    isOutput=False)
    wvd = nc.declare_dram_parameter("wv", [128, 1], f32, isOutput=False)
    erowd = nc.declare_dram_parameter("erow", [1, NIMG], f32, isOutput=False)
    wtd = nc.declare_dram_parameter("wt", [64, 3 * 32], bf16, isOutput=False)
    w0rd = nc.declare_dram_parameter("w0r", [64, 3 * G * 32], bf16,
                                     isOutput=False)
    w1rd = nc.declare_dram_parameter("w1r", [64, 3 * G * 32], bf16,
                                     isOutput=False)
    onesd = nc.declare_dram_parameter("ones32", [32, 1], f32, isOutput=False)
    outd = nc.declare_dram_parameter("out", [NIMG, 32, 32], f32, isOutput=True)

    with TileContext(nc) as tc:
        with (
            tc.tile_pool(name="const", bufs=1) as cpool,
            tc.tile_pool(name="g", bufs=1) as gpool,
            tc.tile_pool(name="xt", bufs=2) as xpool,
            tc.tile_pool(name="xs", bufs=2) as spool,
            tc.tile_pool(name="vsb", bufs=2) as vpool,
            tc.tile_pool(name="pp", bufs=2) as ppool,
            tc.tile_pool(name="fin", bufs=1) as opool,
            tc.tile_pool(name="psv", bufs=1, space="PSUM") as psv,
            tc.tile_pool(name="psb", bufs=1, space="PSUM") as psb,
            tc.tile_pool(name="ps3", bufs=1, space="PSUM") as ps3,
        ):
            # ---- load constants ----
            qt = cpool.tile([128, 6 * 12 * 128], bf16, tag="qt")
            nc.sync.dma_start(qt[:], qtd[:])
            ab = cpool.tile([128, 3 * 256], bf16, tag="ab")
            bb = cpool.tile([128, 3 * 256], bf16, tag="bb")
            nc.sync.dma_start(ab[:], abd[:])
            nc.sync.dma_start(bb[:], bbd[:])
            wv0 = cpool.tile([128, 1], f32, tag="wv0")
            erow0 = cpool.tile([1, NIMG], f32, tag="erow0")
            nc.sync.dma_start(wv0[:], wvd[:])
            nc.sync.dma_start(erow0[:], erowd[:])
            wt = cpool.tile([64, 3 * 32], bf16, tag="wt")
            w0r = cpool.tile([64, 3 * G * 32], bf16, tag="w0r")
            w1r = cpool.tile([64, 3 * G * 32], bf16, tag="w1r")
            nc.sync.dma_start(wt[:], wtd[:])
            nc.sync.dma_start(w0r[:], w0rd[:])
            nc.sync.dma_start(w1r[:], w1rd[:])
            ones0 = cpool.tile([32, 1], f32, tag="ones0")
            nc.sync.dma_start(ones0[:], onesd[:])
            # matmul lhsT wants a single producer sem: copy via one engine
            ones32 = cpool.tile([32, 1], f32, tag="ones32")
            nc.vector.tensor_copy(ones32[:], ones0[:])
            # erow broadcast to all partitions (for outer product via DVE)
            erow_b = gpool.tile([128, NIMG], f32, tag="erow_b")
            nc.gpsimd.partition_broadcast(erow_b[:], erow0[:])

            # ---- batched g vectors (tile-0 layout only; tile 1 reuses them
            # via the partition-reversed consts): gcos/gsin [128, NIMG]
            gcos = gpool.tile([128, NIMG], f32, tag="gcos")
            gsin = gpool.tile([128, NIMG], f32, tag="gsin")
            pg = gpool.tile([128, NIMG], f32, tag="pg")
            nc.vector.tensor_scalar_mul(pg[:], erow_b[:], wv0[:, 0:1])
            gm = gpool.tile([128, NIMG], f32, tag="gm")
            gmc = gpool.tile([128, NIMG], f32, tag="gmc")
            ua = gpool.tile([128, NIMG], f32, tag="ua")
            ub = gpool.tile([128, NIMG], f32, tag="ub")
            nc.vector.tensor_scalar_add(ua[:], pg[:], 256.0)
            nc.vector.tensor_scalar_add(ub[:], pg[:], 256.25)
            ui = gpool.tile([128, NIMG], mybir.dt.int32, tag="ui")
            uf = gpool.tile([128, NIMG], f32, tag="uf")
            nc.vector.tensor_copy(ui[:], ua[:])
            nc.vector.tensor_copy(uf[:], ui[:])
            nc.vector.tensor_sub(gm[:], ua[:], uf[:])
            nc.vector.tensor_copy(ui[:], ub[:])
            nc.vector.tensor_copy(uf[:], ui[:])
            nc.vector.tensor_sub(gmc[:], ub[:], uf[:])
            nc.scalar.activation(gsin[:], gm[:], AF.Sin, scale=TWO_PI)
            nc.scalar.activation(gcos[:], gmc[:], AF.Sin, scale=TWO_PI)

            o_all = opool.tile([32, NIMG * 32], f32, tag="o_all")
            o3 = o_all[:].rearrange("q (j p) -> q j p", p=32)

            # ---- per-group stages; software-pipelined emission so the PE
            # stream never waits on the elementwise chain of the same group.
            ctx = {}

            def group_params(grp):
                f = grp // 6
                i = (grp // 2) % 3
                h = grp % 2
                # image j indices: j = f*48 + b*3 + i, b = h*8 + m
                j0 = f * 48 + (h * G) * 3 + i
                return f, i, j0

            def build_x(grp):
                # xt [128, G*256]; per image m cols =
                #   [kt0: Xr(64)|Xi(64) | kt1(rev-partition): Xr|Xi]
                f, i, j0 = group_params(grp)
                xt = xpool.tile([128, G * 256], bf16, tag="xt")
                csl = slice(i * 256, (i + 1) * 256)
                for m in range(G):
                    j = j0 + 3 * m
                    gc = gcos[:, j: j + 1]
                    gs = gsin[:, j: j + 1]
                    msl = slice(m * 256, (m + 1) * 256)
                    t1 = spool.tile([128, 256], bf16, tag="t1")
                    t2 = spool.tile([128, 256], bf16, tag="t2")
                    nc.vector.tensor_scalar_mul(t1[:], ab[:, csl], gc)
                    nc.vector.tensor_scalar_mul(t2[:], bb[:, csl], gs)
                    nc.vector.tensor_add(xt[:, msl], t1[:], t2[:])
                ctx[grp] = {"xt": xt}

            def stage_a(grp):
                # V[part][mt] [128, G*64] f32 psum
                # Vr = Qr Xr + Qin Xi ; Vi = Qi Xr + Qr Xi
                f, i, j0 = group_params(grp)
                p6 = f * 3 + i
                xt = ctx[grp]["xt"]
                xt3 = xt[:].rearrange("p (m c) -> p m c", c=256)
                xr = [xt3[:, :, kt * 128: kt * 128 + 64] for kt in range(2)]
                xi = [xt3[:, :, kt * 128 + 64: kt * 128 + 128]
                      for kt in range(2)]

                def qtile(var, kt, mt):
                    # var: 0=QrT 1=QiT 2=QinT ; tile [128,128]
                    base = p6 * 12 * 128 + (var * 4 + kt * 2 + mt) * 128
                    return qt[:, base: base + 128]

                vps = []
                for part in range(2):  # 0 = Vr, 1 = Vi
                    for mt in range(2):
                        vp = psv.tile([128, G * 64], f32, tag=f"v{part}{mt}")
                        vps.append(vp)
                        for kt in range(2):
                            v1 = 0 if part == 0 else 1
                            v2 = 2 if part == 0 else 0
                            nc.tensor.matmul(vp[:], qtile(v1, kt, mt), xr[kt],
                                             start=(kt == 0), stop=False)
                            nc.tensor.matmul(vp[:], qtile(v2, kt, mt), xi[kt],
                                             start=False, stop=(kt == 1))
                ctx[grp]["vps"] = vps

            def vcopy(grp):
                # V PSUM -> SBUF bf16 on Act
                # vsb[mt] [128, G*128]: per image m cols = [Vr_m(64)|Vi_m(64)]
                vps = ctx[grp]["vps"]
                vsbs = []
                for mt in range(2):
                    vsb = vpool.tile([128, G * 128], bf16, tag=f"vsb{mt}")
                    vsbs.append(vsb)
                    v3 = vsb[:].rearrange("p (m c) -> p m c", c=128)
                    nc.scalar.copy(v3[:, :, 0:64], vps[0 + mt][:])
                    nc.scalar.copy(v3[:, :, 64:128], vps[2 + mt][:])
                ctx[grp]["vsbs"] = vsbs

            def stage_b(grp):
                # per image pmB [128,128] of 4 products
                # lhsT = [Vr_mt m | Vi_mt m] (contiguous), rhs = xt kt-slice
                xt = ctx[grp]["xt"]
                vsbs = ctx[grp]["vsbs"]
                pmb = psb.tile([128, G * 128], f32, tag="pmb")
                for m in range(G):
                    for mt in range(2):
                        nc.tensor.matmul(
                            pmb[:, m * 128:(m + 1) * 128],
                            vsbs[mt][:, m * 128:(m + 1) * 128],
                            xt[:, m * 256 + mt * 128: m * 256 + mt * 128 + 128],
                            start=(mt == 0), stop=(mt == 1))
                ctx[grp]["pmb"] = pmb

            def chain(grp):
                # pmb -> psf -> blended a1 (Act / DVE / Pool)
                f, i, j0 = group_params(grp)
                pmb = ctx[grp]["pmb"]
                pmb3 = pmb[:].rearrange("p (m x c) -> p m x c", m=G, x=2)
                # bottom half (Vi-products) -> SBUF bf16 at partition 0
                pmbot = ppool.tile([64, G * 128], bf16, tag="pmbot")
                nc.scalar.copy(pmbot[:], pmb[64:128, :])
                pmt3 = pmbot[:].rearrange("p (m x c) -> p m x c", m=G, x=2)
                # d1 = P11 - P22, d2 = P12 + P21  [64, G*64] bf16 (DVE)
                d1 = ppool.tile([64, G * 64], bf16, tag="d1")
                d2 = ppool.tile([64, G * 64], bf16, tag="d2")
                nc.vector.scalar_tensor_tensor(
                    d1[:], pmb3[0:64, :, 0, :], 1.0, pmt3[:, :, 1, :],
                    op0=OP.mult, op1=OP.subtract)
                nc.vector.scalar_tensor_tensor(
                    d2[:], pmb3[0:64, :, 1, :], 1.0, pmt3[:, :, 0, :],
                    op0=OP.mult, op1=OP.add)
                # squares (Act + Pool) + add (Pool): psf^T = d1^2 + d2^2
                sq1 = ppool.tile([64, G * 64], bf16, tag="sq1")
                sq2 = ppool.tile([64, G * 64], bf16, tag="sq2")
                nc.scalar.activation(sq1[:], d1[:], AF.Square)
                nc.gpsimd.tensor_mul(sq2[:], d2[:], d2[:])
                psf = ppool.tile([64, G * 64], bf16, tag="psf")
                nc.gpsimd.tensor_add(psf[:], sq1[:], sq2[:])
                # y-side 2-tap blend -> a1 [64, G*32]
                # y-taps stored [even-block(32) | odd-block(32)] per image
                psf3 = psf[:].rearrange("p (m two a) -> p m two a",
                                        m=G, two=2)
                wsl = slice(i * G * 32, (i + 1) * G * 32)
                w0v = w0r[:, wsl].rearrange("p (m a) -> p m a", m=G)
                w1v = w1r[:, wsl].rearrange("p (m a) -> p m a", m=G)
                ea = ppool.tile([64, G * 32], bf16, tag="ea")
                eb = ppool.tile([64, G * 32], bf16, tag="eb")
                a1 = ppool.tile([64, G * 32], bf16, tag="a1")
                ea3 = ea[:].rearrange("p (m a) -> p m a", m=G)
                eb3 = eb[:].rearrange("p (m a) -> p m a", m=G)
                nc.gpsimd.tensor_mul(ea3, psf3[:, :, 0, :], w0v)
                nc.gpsimd.tensor_mul(eb3, psf3[:, :, 1, :], w1v)
                nc.gpsimd.tensor_add(a1[:], ea[:], eb[:])
                ctx[grp]["a1"] = a1

            def stage3(grp):
                # x-side via matmul: [32(q), G*32(p)] + scatter into o_all
                f, i, j0 = group_params(grp)
                a1 = ctx[grp]["a1"]
                pm3 = ps3.tile([32, G * 32], f32, tag="pm3")
                nc.tensor.matmul(pm3[:], wt[:, i * 32:(i + 1) * 32], a1[:],
                                 start=True, stop=True)
                nc.scalar.copy(o3[:, j0: j0 + 3 * (G - 1) + 1: 3, :],
                               pm3[:].rearrange("q (m p) -> q m p", m=G))
                del ctx[grp]

            for grp in range(NGRP):
                build_x(grp)
                stage_a(grp)
                vcopy(grp)
                if grp >= 1:
                    stage_b(grp - 1)
                    chain(grp - 1)
                if grp >= 2:
                    stage3(grp - 2)
            stage_b(NGRP - 1)
            chain(NGRP - 1)
            stage3(NGRP - 2)
            stage3(NGRP - 1)

            # ---- batched finalize: sums, normalize, block-transpose, out
            csum = opool.tile([32, NIMG], f32, tag="csum")
            nc.vector.tensor_reduce(csum[:], o3, op=OP.add,
                                    axis=mybir.AxisListType.X)
            pcs = ps3.tile([1, NIMG], f32, tag="pcs")
            nc.tensor.matmul(pcs[:], ones32[:], csum[:], start=True, stop=True)
            rec = opool.tile([1, NIMG], f32, tag="rec")
            nc.vector.tensor_scalar_add(rec[:], pcs[:], 1e-8)
            nc.vector.reciprocal(rec[:], rec[:])
            recb = opool.tile([32, NIMG], f32, tag="recb")
            nc.gpsimd.partition_broadcast(recb[:], rec[:])
            t_all = opool.tile([32, NIMG * 32], f32, tag="t_all")
            nc.vector.transpose(t_all[:], o_all[:])
            for j in range(NIMG):
                jsl = slice(j * 32, (j + 1) * 32)
                nc.vector.tensor_scalar_mul(t_all[:, jsl], t_all[:, jsl],
                                            recb[:, j:j + 1])
            nc.sync.dma_start(outd[:].rearrange("j p q -> p j q"),
                              t_all[:].rearrange("p (j q) -> p j q", q=32))
    nc.compile()
    return nc


_CACHE = {}


def _get_nc():
    if "nc" not in _CACHE:
        _CACHE["nc"] = build_nc()
    return _CACHE["nc"]


def kernel(d_obj, current_focus_dist_0, current_focus_dist_90,
           zernike_0, zernike_90, zernike_basis, aperture, wavelengths):
    from concourse.bass_utils import run_bass_kernel_spmd
    import ml_dtypes
    bf = ml_dtypes.bfloat16

    d_obj = np.asarray(d_obj, np.float32)
    zernike_0 = np.asarray(zernike_0, np.float32)
    zernike_90 = np.asarray(zernike_90, np.float32)
    basis = np.asarray(zernike_basis, np.float32)
    aperture = np.asarray(aperture, np.float32)
    lam = np.asarray(wavelengths, np.float32)
    f0 = float(current_focus_dist_0)
    f90 = float(current_focus_dist_90)

    Fr_full, Fi_full, csel, wt0, wt1 = _host_consts(lam)

    # Q planes -> transposed tiles [QrT | QiT | QinT] per (f,i)
    O = np.tensordot(np.stack([zernike_0, zernike_90]),
                     basis.reshape(NZ, -1), axes=[[1], [0]])  # [2, 65536]
    O = O.reshape(2, GRID, GRID).astype(np.float64)
    qt = np.empty((128, 6 * 12 * 128), bf)
    for f in range(2):
        for i in range(3):
            ph = 2.0 * np.pi * O[f] / float(lam[i])
            Qr = (aperture * np.cos(ph))
            Qi = (aperture * np.sin(ph))
            p6 = f * 3 + i
            for var, Qm in enumerate((Qr, Qi, -Qi)):
                QT = Qm.T.astype(bf)        # [v, v']
                for kt in range(2):
                    for mt in range(2):
                        idx = p6 * 12 + var * 4 + kt * 2 + mt
                        T = QT[kt * 128:(kt + 1) * 128,
                               mt * 128:(mt + 1) * 128]
                        # tile-1 halves of X / V are partition-reversed
                        if kt == 1:
                            T = T[::-1, :]
                        if mt == 1:
                            T = T[:, ::-1]
                        qt[:, idx * 128:(idx + 1) * 128] = T

    # X-build consts: per channel i: [A_t0 | flipud(A_t1)], A = [FcrT|FciT],
    # B = [-FciT|FcrT] (tile-1 partition-reversed to reuse tile-0 g scalars)
    ab = np.empty((128, 3 * 256), bf)
    bb = np.empty((128, 3 * 256), bf)
    for i in range(3):
        FcrT = Fr_full[:, csel[i]].astype(np.float32)   # [256, 64]
        FciT = Fi_full[:, csel[i]].astype(np.float32)
        A = np.concatenate([FcrT, FciT], 1)             # [256, 128]
        B = np.concatenate([-FciT, FcrT], 1)
        sl = slice(i * 256, (i + 1) * 256)
        ab[:, sl] = np.concatenate([A[0:128], A[128:256][::-1]], 1).astype(bf)
        bb[:, sl] = np.concatenate([B[0:128], B[128:256][::-1]], 1).astype(bf)

    # wv col (tile 0 only): 2*v^2 - 0.5 on the [-1,1] grid, [128, 1]
    lin = np.linspace(-1.0, 1.0, GRID)
    wv = np.ascontiguousarray(
        (2.0 * lin * lin - 0.5).astype(np.float32)[0:128, None])

    # W^T (x-blend) and y-tap weight mats (row-replicated, per-image tiled)
    wt = np.zeros((64, 3 * 32), np.float32)
    w0r = np.zeros((64, 3 * G * 32), np.float32)
    w1r = np.zeros((64, 3 * G * 32), np.float32)
    for i in range(3):
        W = np.zeros((32, 64), np.float32)
        for p in range(32):
            W[p, p] = wt0[i, p]
            W[p, 32 + p] = wt1[i, p]
        wt[:, i * 32:(i + 1) * 32] = W.T
        w0r[:, i * G * 32:(i + 1) * G * 32] = \
            np.tile(wt0[i][None, :], (64, G))
        w1r[:, i * G * 32:(i + 1) * G * 32] = \
            np.tile(wt1[i][None, :], (64, G))

    # defocus scalars -> e_j = delta * sqrt(3) / lam
    delta = np.stack([
        F_M ** 2 / (8.0 * F_NUMBER ** 2) * (1.0 / f0 - 1.0 / (d_obj + 1e-8)),
        F_M ** 2 / (8.0 * F_NUMBER ** 2) * (1.0 / f90 - 1.0 / (d_obj + 1e-8)),
    ])  # [2, 128]

    nc = _get_nc()
    in_maps = []
    for c in range(NCORES):
        erow = np.empty((1, NIMG), np.float32)
        for j in range(NIMG):
            f = j // (BPC * 3)
            b = (j // 3) % BPC
            i = j % 3
            erow[0, j] = delta[f, c * BPC + b] * np.sqrt(3.0) / float(lam[i])
        in_maps.append({
            "qt": qt, "ab": ab, "bb": bb,
            "wv": wv, "erow": erow,
            "wt": wt.astype(bf), "w0r": w0r.astype(bf), "w1r": w1r.astype(bf),
            "ones32": np.ones((32, 1), np.float32),
        })
    trace = bool(_CACHE.get("trace"))
    res = run_bass_kernel_spmd(nc, in_maps, list(range(NCORES)), trace=trace)
    _CACHE["last_res"] = res
    outs = res.results
    psf0 = np.empty((BATCH, 3, FOV, FOV), np.float32)
    psf90 = np.empty((BATCH, 3, FOV, FOV), np.float32)
    for c in range(NCORES):
        o = np.asarray(outs[c]["out"]).reshape(2, BPC, 3, FOV, FOV)
        psf0[c * BPC:(c + 1) * BPC] = o[0]
        psf90[c * BPC:(c + 1) * BPC] = o[1]
    return psf0, psf90


# revision 28
# speedup vs baseline: 1.2751x; 1.2751x over previous
"""Trainium2 Bass kernel: differentiable-optics PSF (batch=128, 2 focus, 3 ch).

Math per image (b, f, i): pupil P = Q ∘ (g g^T), Q = A*exp(i*2pi*O_f/lam_i)
(shared per (f,i)), g = exp(i*a*w(v)) per-image separable defocus.
Sampled field taps: field = Fc P Fc^T = (Fc diag(g)) Q (diag(g) Fc^T)
                          = X^T Q^T ... with X = diag(g) Fc^T  [256, 64] cplx.
Stage A:  V = Q X   (lhsT = Q^T tiles, rhs = X)      -> PSUM [256, 64] cplx
Stage B:  per image one PSUM tile [128,128] of all 4 products
          [Vr|Vi]^T [Xr|Xi] ; fieldR^T = P11 - P22', fieldI^T = P12 + P21.
Images grouped 8 per (f, i) so stage A batches N=512 and post-field
element-wise work runs on [64, 512] tiles.
psf^T -> y-blend (2-tap, elementwise) -> Wx matmul -> [32, (img,32)],
batched normalize + 32x32 block transpose at the end (as v1).
"""
import numpy as np
from math import factorial

GRID = 256
FOV = 32
NZ = 15
F_MM = 25.0
F_NUMBER = 2.0
PIXEL_SIZE = 3.45e-6
F_M = F_MM * 1e-3
PUPIL_DIAM = F_M / F_NUMBER
BATCH = 128
NCORES = 8
BPC = BATCH // NCORES          # batch per core (16)
NIMG = BPC * 2 * 3             # images per core (96)
G = 8                          # images per group
NGRP = NIMG // G               # 12 groups per core


def _noll_to_nm(j):
    n = 0
    k = j - 1
    while k > n:
        n += 1
        k -= n
    m = (-1) ** j * ((n % 2) + 2 * ((k + ((n + 1) % 2)) // 2))
    return n, m


def _zernike(n, m, r, theta):
    am = abs(m)
    R = np.zeros_like(r)
    for s in range((n - am) // 2 + 1):
        c = ((-1) ** s * factorial(n - s)
             / (factorial(s) * factorial((n + am) // 2 - s)
                * factorial((n - am) // 2 - s)))
        R = R + c * r ** (n - 2 * s)
    norm = np.sqrt(n + 1) if m == 0 else np.sqrt(2 * (n + 1))
    ang = np.cos(am * theta) if m >= 0 else np.sin(am * theta)
    return np.where(r <= 1.0, norm * R * ang, 0.0)


def _host_consts(wavelengths):
    """Input-independent structural constants (DFT/sampling matrices)."""
    N = GRID
    # shifted DFT: field = Ft X Ft, Ft[a,b] = F[(a+128)%256,(b+128)%256]
    idx = (np.arange(N) + N // 2) % N
    jk = np.outer(idx, idx).astype(np.float64)
    ang = -2.0 * np.pi * jk / N
    Fr_full = np.cos(ang)
    Fi_full = np.sin(ang)

    csel = np.zeros((3, 64), np.int64)      # tap rows per channel
    wt0 = np.zeros((3, 32), np.float32)     # tap weights
    wt1 = np.zeros((3, 32), np.float32)
    for i in range(3):
        lam = float(wavelengths[i])
        zoom = PIXEL_SIZE * FOV * PUPIL_DIAM / (lam * F_M * GRID)
        g1 = (np.arange(FOV, dtype=np.float32) / np.float32(FOV - 1)
              * np.float32(2.0 * zoom) - np.float32(zoom))
        x = ((g1 + 1.0) * GRID - 1.0) * 0.5
        x0 = np.floor(x)
        tx = (x - x0).astype(np.float32)
        x0 = x0.astype(np.int64)
        csel[i, 0:32] = x0
        csel[i, 32:64] = x0 + 1
        wt0[i] = 1.0 - tx
        wt1[i] = tx
    return Fr_full, Fi_full, csel, wt0, wt1


def build_nc():
    import concourse.bass as bass
    import concourse.bacc as bacc
    import concourse.mybir as mybir
    from concourse.tile import TileContext

    f32 = mybir.dt.float32
    bf16 = mybir.dt.bfloat16
    AF = mybir.ActivationFunctionType
    OP = mybir.AluOpType
    TWO_PI = float(2.0 * np.pi)

    nc = bacc.Bacc("TRN2", target_bir_lowering=False)
    # device inputs (per core)
    # Q^T tiles: per (f,i) p6: [QrT(ktmt 4) | QiT(4) | QinT(4)] each [128,128]
    qtd = nc.declare_dram_parameter("qt", [128, 6 * 12 * 128], bf16,
                                    isOutput=False)
    # X-build consts per channel i: [A_t0 | flip(A_t1)] with A = [FcrT|FciT],
    # B = [-FciT|FcrT].  Tile-1 halves are partition-reversed so tile-0 g
    # scalars apply (w(v) is symmetric); Q^T tiles are flipped to match.
    abd = nc.declare_dram_parameter("ab", [128, 3 * 256], bf16,
                                    isOutput=False)
    bbd = nc.declare_dram_parameter("bb", [128, 3 * 256], bf16,
                                    isOutput=False)
    wvd = nc.declare_dram_parameter("wv", [128, 1], f32, isOutput=False)
    erowd = nc.declare_dram_parameter("erow", [1, NIMG], f32, isOutput=False)
    wtd = nc.declare_dram_parameter("wt", [64, 3 * 32], bf16, isOutput=False)
    w0rd = nc.declare_dram_parameter("w0r", [64, 3 * G * 32], bf16,
                                     isOutput=False)
    w1rd = nc.declare_dram_parameter("w1r", [64, 3 * G * 32], bf16,
                                     isOutput=False)
    onesd = nc.declare_dram_parameter("ones32", [32, 1], f32, isOutput=False)
    outd = nc.declare_dram_parameter("out", [NIMG, 32, 32], f32, isOutput=True)

    with TileContext(nc) as tc:
        with (
            tc.tile_pool(name="const", bufs=1) as cpool,
            tc.tile_pool(name="g", bufs=1) as gpool,
            tc.tile_pool(name="xt", bufs=4) as xpool,
            tc.tile_pool(name="xs", bufs=2) as spool,
            tc.tile_pool(name="vsb", bufs=2) as vpool,
            tc.tile_pool(name="pp", bufs=2) as ppool,
            tc.tile_pool(name="fin", bufs=1) as opool,
            tc.tile_pool(name="psv", bufs=1, space="PSUM") as psv,
            tc.tile_pool(name="psb", bufs=1, space="PSUM") as psb,
            tc.tile_pool(name="ps3", bufs=1, space="PSUM") as ps3,
        ):
            # ---- load constants ----
            qt = cpool.tile([128, 6 * 12 * 128], bf16, tag="qt")
            nc.sync.dma_start(qt[:], qtd[:])
            ab = cpool.tile([128, 3 * 256], bf16, tag="ab")
            bb = cpool.tile([128, 3 * 256], bf16, tag="bb")
            nc.sync.dma_start(ab[:], abd[:])
            nc.sync.dma_start(bb[:], bbd[:])
            wv0 = cpool.tile([128, 1], f32, tag="wv0")
            erow0 = cpool.tile([1, NIMG], f32, tag="erow0")
            nc.sync.dma_start(wv0[:], wvd[:])
            nc.sync.dma_start(erow0[:], erowd[:])
            wt = cpool.tile([64, 3 * 32], bf16, tag="wt")
            w0r = cpool.tile([64, 3 * G * 32], bf16, tag="w0r")
            w1r = cpool.tile([64, 3 * G * 32], bf16, tag="w1r")
            nc.sync.dma_start(wt[:], wtd[:])
            nc.sync.dma_start(w0r[:], w0rd[:])
            nc.sync.dma_start(w1r[:], w1rd[:])
            ones0 = cpool.tile([32, 1], f32, tag="ones0")
            nc.sync.dma_start(ones0[:], onesd[:])
            # matmul lhsT wants a single producer sem: copy via one engine
            ones32 = cpool.tile([32, 1], f32, tag="ones32")
            nc.vector.tensor_copy(ones32[:], ones0[:])
            # erow broadcast to all partitions (for outer product via DVE)
            erow_b = gpool.tile([128, NIMG], f32, tag="erow_b")
            nc.gpsimd.partition_broadcast(erow_b[:], erow0[:])

            # ---- batched g vectors (tile-0 layout only; tile 1 reuses them
            # via the partition-reversed consts): gcos/gsin [128, NIMG]
            gcos = gpool.tile([128, NIMG], f32, tag="gcos")
            gsin = gpool.tile([128, NIMG], f32, tag="gsin")
            pg = gpool.tile([128, NIMG], f32, tag="pg")
            nc.vector.tensor_scalar_mul(pg[:], erow_b[:], wv0[:, 0:1])
            gm = gpool.tile([128, NIMG], f32, tag="gm")
            gmc = gpool.tile([128, NIMG], f32, tag="gmc")
            ua = gpool.tile([128, NIMG], f32, tag="ua")
            ub = gpool.tile([128, NIMG], f32, tag="ub")
            nc.vector.tensor_scalar_add(ua[:], pg[:], 256.0)
            nc.vector.tensor_scalar_add(ub[:], pg[:], 256.25)
            ui = gpool.tile([128, NIMG], mybir.dt.int32, tag="ui")
            uf = gpool.tile([128, NIMG], f32, tag="uf")
            nc.vector.tensor_copy(ui[:], ua[:])
            nc.vector.tensor_copy(uf[:], ui[:])
            nc.vector.tensor_sub(gm[:], ua[:], uf[:])
            nc.vector.tensor_copy(ui[:], ub[:])
            nc.vector.tensor_copy(uf[:], ui[:])
            nc.vector.tensor_sub(gmc[:], ub[:], uf[:])
            nc.scalar.activation(gsin[:], gm[:], AF.Sin, scale=TWO_PI)
            nc.scalar.activation(gcos[:], gmc[:], AF.Sin, scale=TWO_PI)

            o_all = opool.tile([32, NIMG * 32], f32, tag="o_all")
            o3 = o_all[:].rearrange("q (j p) -> q j p", p=32)

            # ---- per-group stages; software-pipelined emission so the PE
            # stream never waits on the elementwise chain of the same group.
            ctx = {}

            def group_params(grp):
                f = grp // 6
                i = (grp // 2) % 3
                h = grp % 2
                # image j indices: j = f*48 + b*3 + i, b = h*8 + m
                j0 = f * 48 + (h * G) * 3 + i
                return f, i, j0

            def build_x(grp):
                # xt [128, G*256]; per image m cols =
                #   [kt0: Xr(64)|Xi(64) | kt1(rev-partition): Xr|Xi]
                f, i, j0 = group_params(grp)
                xt = xpool.tile([128, G * 256], bf16, tag="xt")
                csl = slice(i * 256, (i + 1) * 256)
                for m in range(G):
                    j = j0 + 3 * m
                    gc = gcos[:, j: j + 1]
                    gs = gsin[:, j: j + 1]
                    msl = slice(m * 256, (m + 1) * 256)
                    t1 = spool.tile([128, 256], bf16, tag="t1")
                    t2 = spool.tile([128, 256], bf16, tag="t2")
                    nc.vector.tensor_scalar_mul(t1[:], ab[:, csl], gc)
                    nc.vector.tensor_scalar_mul(t2[:], bb[:, csl], gs)
                    nc.vector.tensor_add(xt[:, msl], t1[:], t2[:])
                ctx[grp] = {"xt": xt}

            def stage_a(grp):
                # V[part][mt] [128, G*64] f32 psum
                # Vr = Qr Xr + Qin Xi ; Vi = Qi Xr + Qr Xi
                f, i, j0 = group_params(grp)
                p6 = f * 3 + i
                xt = ctx[grp]["xt"]
                xt3 = xt[:].rearrange("p (m c) -> p m c", c=256)
                xr = [xt3[:, :, kt * 128: kt * 128 + 64] for kt in range(2)]
                xi = [xt3[:, :, kt * 128 + 64: kt * 128 + 128]
                      for kt in range(2)]

                def qtile(var, kt, mt):
                    # var: 0=QrT 1=QiT 2=QinT ; tile [128,128]
                    base = p6 * 12 * 128 + (var * 4 + kt * 2 + mt) * 128
                    return qt[:, base: base + 128]

                vps = []
                for part in range(2):  # 0 = Vr, 1 = Vi
                    for mt in range(2):
                        vp = psv.tile([128, G * 64], f32, tag=f"v{part}{mt}")
                        vps.append(vp)
                        for kt in range(2):
                            v1 = 0 if part == 0 else 1
                            v2 = 2 if part == 0 else 0
                            nc.tensor.matmul(vp[:], qtile(v1, kt, mt), xr[kt],
                                             start=(kt == 0), stop=False)
                            nc.tensor.matmul(vp[:], qtile(v2, kt, mt), xi[kt],
                                             start=False, stop=(kt == 1))
                ctx[grp]["vps"] = vps

            def vcopy(grp):
                # V PSUM -> SBUF bf16 on Act
                # vsb[mt] [128, G*128]: per image m cols = [Vr_m(64)|Vi_m(64)]
                vps = ctx[grp]["vps"]
                vsbs = []
                for mt in range(2):
                    vsb = vpool.tile([128, G * 128], bf16, tag=f"vsb{mt}")
                    vsbs.append(vsb)
                    v3 = vsb[:].rearrange("p (m c) -> p m c", c=128)
                    nc.scalar.copy(v3[:, :, 0:64], vps[0 + mt][:])
                    nc.scalar.copy(v3[:, :, 64:128], vps[2 + mt][:])
                ctx[grp]["vsbs"] = vsbs

            def stage_b(grp):
                # per image pmB [128,128] of 4 products
                # lhsT = [Vr_mt m | Vi_mt m] (contiguous), rhs = xt kt-slice
                xt = ctx[grp]["xt"]
                vsbs = ctx[grp]["vsbs"]
                pmb = psb.tile([128, G * 128], f32, tag="pmb")
                for m in range(G):
                    for mt in range(2):
                        nc.tensor.matmul(
                            pmb[:, m * 128:(m + 1) * 128],
                            vsbs[mt][:, m * 128:(m + 1) * 128],
                            xt[:, m * 256 + mt * 128: m * 256 + mt * 128 + 128],
                            start=(mt == 0), stop=(mt == 1))
                ctx[grp]["pmb"] = pmb

            def chain(grp):
                # pmb -> psf -> blended a1 (Act / DVE / Pool)
                f, i, j0 = group_params(grp)
                pmb = ctx[grp]["pmb"]
                pmb3 = pmb[:].rearrange("p (m x c) -> p m x c", m=G, x=2)
                # bottom half (Vi-products) -> SBUF bf16 at partition 0
                pmbot = ppool.tile([64, G * 128], bf16, tag="pmbot")
                nc.scalar.copy(pmbot[:], pmb[64:128, :])
                pmt3 = pmbot[:].rearrange("p (m x c) -> p m x c", m=G, x=2)
                # d1 = P11 - P22, d2 = P12 + P21  [64, G*64] bf16 (DVE)
                d1 = ppool.tile([64, G * 64], bf16, tag="d1")
                d2 = ppool.tile([64, G * 64], bf16, tag="d2")
                nc.vector.scalar_tensor_tensor(
                    d1[:], pmb3[0:64, :, 0, :], 1.0, pmt3[:, :, 1, :],
                    op0=OP.mult, op1=OP.subtract)
                nc.vector.scalar_tensor_tensor(
                    d2[:], pmb3[0:64, :, 1, :], 1.0, pmt3[:, :, 0, :],
                    op0=OP.mult, op1=OP.add)
                # squares (Act + Pool) + add (Pool): psf^T = d1^2 + d2^2
                sq1 = ppool.tile([64, G * 64], bf16, tag="sq1")
                sq2 = ppool.tile([64, G * 64], bf16, tag="sq2")
                nc.scalar.activation(sq1[:], d1[:], AF.Square)
                nc.gpsimd.tensor_mul(sq2[:], d2[:], d2[:])
                psf = ppool.tile([64, G * 64], bf16, tag="psf")
                nc.gpsimd.tensor_add(psf[:], sq1[:], sq2[:])
                # y-side 2-tap blend -> a1 [64, G*32]
                # y-taps stored [even-block(32) | odd-block(32)] per image
                psf3 = psf[:].rearrange("p (m two a) -> p m two a",
                                        m=G, two=2)
                wsl = slice(i * G * 32, (i + 1) * G * 32)
                w0v = w0r[:, wsl].rearrange("p (m a) -> p m a", m=G)
                w1v = w1r[:, wsl].rearrange("p (m a) -> p m a", m=G)
                ea = ppool.tile([64, G * 32], bf16, tag="ea")
                eb = ppool.tile([64, G * 32], bf16, tag="eb")
                a1 = ppool.tile([64, G * 32], bf16, tag="a1")
                ea3 = ea[:].rearrange("p (m a) -> p m a", m=G)
                eb3 = eb[:].rearrange("p (m a) -> p m a", m=G)
                nc.gpsimd.tensor_mul(ea3, psf3[:, :, 0, :], w0v)
                nc.gpsimd.tensor_mul(eb3, psf3[:, :, 1, :], w1v)
                nc.gpsimd.tensor_add(a1[:], ea[:], eb[:])
                ctx[grp]["a1"] = a1

            def stage3(grp):
                # x-side via matmul: [32(q), G*32(p)] + scatter into o_all
                f, i, j0 = group_params(grp)
                a1 = ctx[grp]["a1"]
                pm3 = ps3.tile([32, G * 32], f32, tag="pm3")
                nc.tensor.matmul(pm3[:], wt[:, i * 32:(i + 1) * 32], a1[:],
                                 start=True, stop=True)
                nc.scalar.copy(o3[:, j0: j0 + 3 * (G - 1) + 1: 3, :],
                               pm3[:].rearrange("q (m p) -> q m p", m=G))
                del ctx[grp]

            # X is built 2 groups ahead so the PE never waits on the DVE
            # (A(g) needs the COMPLETE xt(g); d(g-1) sits ahead of X(g+2)
            # in the DVE stream but has 2 iterations of slack).
            build_x(0)
            build_x(1)
            for grp in range(NGRP):
                if grp + 2 < NGRP:
                    build_x(grp + 2)
                stage_a(grp)
                vcopy(grp)
                if grp >= 1:
                    stage_b(grp - 1)
                    chain(grp - 1)
                if grp >= 2:
                    stage3(grp - 2)
            stage_b(NGRP - 1)
            chain(NGRP - 1)
            stage3(NGRP - 2)
            stage3(NGRP - 1)

            # ---- batched finalize: sums, normalize, block-transpose, out
            csum = opool.tile([32, NIMG], f32, tag="csum")
            nc.vector.tensor_reduce(csum[:], o3, op=OP.add,
                                    axis=mybir.AxisListType.X)
            pcs = ps3.tile([1, NIMG], f32, tag="pcs")
            nc.tensor.matmul(pcs[:], ones32[:], csum[:], start=True, stop=True)
            rec = opool.tile([1, NIMG], f32, tag="rec")
            nc.vector.tensor_scalar_add(rec[:], pcs[:], 1e-8)
            nc.vector.reciprocal(rec[:], rec[:])
            recb = opool.tile([32, NIMG], f32, tag="recb")
            nc.gpsimd.partition_broadcast(recb[:], rec[:])
            t_all = opool.tile([32, NIMG * 32], f32, tag="t_all")
            nc.vector.transpose(t_all[:], o_all[:])
            for j in range(NIMG):
                jsl = slice(j * 32, (j + 1) * 32)
                nc.vector.tensor_scalar_mul(t_all[:, jsl], t_all[:, jsl],
                                            recb[:, j:j + 1])
            nc.sync.dma_start(outd[:].rearrange("j p q -> p j q"),
                              t_all[:].rearrange("p (j q) -> p j q", q=32))
    nc.compile()
    return nc


_CACHE = {}


def _get_nc():
    if "nc" not in _CACHE:
        _CACHE["nc"] = build_nc()
    return _CACHE["nc"]


def kernel(d_obj, current_focus_dist_0, current_focus_dist_90,
           zernike_0, zernike_90, zernike_basis, aperture, wavelengths):
    from concourse.bass_utils import run_bass_kernel_spmd
    import ml_dtypes
    bf = ml_dtypes.bfloat16

    d_obj = np.asarray(d_obj, np.float32)
    zernike_0 = np.asarray(zernike_0, np.float32)
    zernike_90 = np.asarray(zernike_90, np.float32)
    basis = np.asarray(zernike_basis, np.float32)
    aperture = np.asarray(aperture, np.float32)
    lam = np.asarray(wavelengths, np.float32)
    f0 = float(current_focus_dist_0)
    f90 = float(current_focus_dist_90)

    Fr_full, Fi_full, csel, wt0, wt1 = _host_consts(lam)

    # Q planes -> transposed tiles [QrT | QiT | QinT] per (f,i)
    O = np.tensordot(np.stack([zernike_0, zernike_90]),
                     basis.reshape(NZ, -1), axes=[[1], [0]])  # [2, 65536]
    O = O.reshape(2, GRID, GRID).astype(np.float64)
    qt = np.empty((128, 6 * 12 * 128), bf)
    for f in range(2):
        for i in range(3):
            ph = 2.0 * np.pi * O[f] / float(lam[i])
            Qr = (aperture * np.cos(ph))
            Qi = (aperture * np.sin(ph))
            p6 = f * 3 + i
            for var, Qm in enumerate((Qr, Qi, -Qi)):
                QT = Qm.T.astype(bf)        # [v, v']
                for kt in range(2):
                    for mt in range(2):
                        idx = p6 * 12 + var * 4 + kt * 2 + mt
                        T = QT[kt * 128:(kt + 1) * 128,
                               mt * 128:(mt + 1) * 128]
                        # tile-1 halves of X / V are partition-reversed
                        if kt == 1:
                            T = T[::-1, :]
                        if mt == 1:
                            T = T[:, ::-1]
                        qt[:, idx * 128:(idx + 1) * 128] = T

    # X-build consts: per channel i: [A_t0 | flipud(A_t1)], A = [FcrT|FciT],
    # B = [-FciT|FcrT] (tile-1 partition-reversed to reuse tile-0 g scalars)
    ab = np.empty((128, 3 * 256), bf)
    bb = np.empty((128, 3 * 256), bf)
    for i in range(3):
        FcrT = Fr_full[:, csel[i]].astype(np.float32)   # [256, 64]
        FciT = Fi_full[:, csel[i]].astype(np.float32)
        A = np.concatenate([FcrT, FciT], 1)             # [256, 128]
        B = np.concatenate([-FciT, FcrT], 1)
        sl = slice(i * 256, (i + 1) * 256)
        ab[:, sl] = np.concatenate([A[0:128], A[128:256][::-1]], 1).astype(bf)
        bb[:, sl] = np.concatenate([B[0:128], B[128:256][::-1]], 1).astype(bf)

    # wv col (tile 0 only): 2*v^2 - 0.5 on the [-1,1] grid, [128, 1]
    lin = np.linspace(-1.0, 1.0, GRID)
    wv = np.ascontiguousarray(
        (2.0 * lin * lin - 0.5).astype(np.float32)[0:128, None])

    # W^T (x-blend) and y-tap weight mats (row-replicated, per-image tiled)
    wt = np.zeros((64, 3 * 32), np.float32)
    w0r = np.zeros((64, 3 * G * 32), np.float32)
    w1r = np.zeros((64, 3 * G * 32), np.float32)
    for i in range(3):
        W = np.zeros((32, 64), np.float32)
        for p in range(32):
            W[p, p] = wt0[i, p]
            W[p, 32 + p] = wt1[i, p]
        wt[:, i * 32:(i + 1) * 32] = W.T
        w0r[:, i * G * 32:(i + 1) * G * 32] = \
            np.tile(wt0[i][None, :], (64, G))
        w1r[:, i * G * 32:(i + 1) * G * 32] = \
            np.tile(wt1[i][None, :], (64, G))

    # defocus scalars -> e_j = delta * sqrt(3) / lam
    delta = np.stack([
        F_M ** 2 / (8.0 * F_NUMBER ** 2) * (1.0 / f0 - 1.0 / (d_obj + 1e-8)),
        F_M ** 2 / (8.0 * F_NUMBER ** 2) * (1.0 / f90 - 1.0 / (d_obj + 1e-8)),
    ])  # [2, 128]

    nc = _get_nc()
    in_maps = []
    for c in range(NCORES):
        erow = np.empty((1, NIMG), np.float32)
        for j in range(NIMG):
            f = j // (BPC * 3)
            b = (j // 3) % BPC
            i = j % 3
            erow[0, j] = delta[f, c * BPC + b] * np.sqrt(3.0) / float(lam[i])
        in_maps.append({
            "qt": qt, "ab": ab, "bb": bb,
            "wv": wv, "erow": erow,
            "wt": wt.astype(bf), "w0r": w0r.astype(bf), "w1r": w1r.astype(bf),
            "ones32": np.ones((32, 1), np.float32),
        })
    trace = bool(_CACHE.get("trace"))
    res = run_bass_kernel_spmd(nc, in_maps, list(range(NCORES)), trace=trace)
    _CACHE["last_res"] = res
    outs = res.results
    psf0 = np.empty((BATCH, 3, FOV, FOV), np.float32)
    psf90 = np.empty((BATCH, 3, FOV, FOV), np.float32)
    for c in range(NCORES):
        o = np.asarray(outs[c]["out"]).reshape(2, BPC, 3, FOV, FOV)
        psf0[c * BPC:(c + 1) * BPC] = o[0]
        psf90[c * BPC:(c + 1) * BPC] = o[1]
    return psf0, psf90


# revision 33
# speedup vs baseline: 1.3253x; 1.0394x over previous
"""Trainium2 Bass kernel: differentiable-optics PSF (batch=128, 2 focus, 3 ch).

Math per image (b, f, i): pupil P = Q ∘ (g g^T), Q = A*exp(i*2pi*O_f/lam_i)
(shared per (f,i)), g = exp(i*a*w(v)) per-image separable defocus.
Sampled field taps: field = Fc P Fc^T = (Fc diag(g)) Q (diag(g) Fc^T)
                          = X^T Q^T ... with X = diag(g) Fc^T  [256, 64] cplx.
Stage A:  V = Q X   (lhsT = Q^T tiles, rhs = X)      -> PSUM [256, 64] cplx
Stage B:  per image one PSUM tile [128,128] of all 4 products
          [Vr|Vi]^T [Xr|Xi] ; fieldR^T = P11 - P22', fieldI^T = P12 + P21.
Images grouped 8 per (f, i) so stage A batches N=512 and post-field
element-wise work runs on [64, 512] tiles.
psf^T -> y-blend (2-tap, elementwise) -> Wx matmul -> [32, (img,32)],
batched normalize + 32x32 block transpose at the end (as v1).
"""
import numpy as np
from math import factorial

GRID = 256
FOV = 32
NZ = 15
F_MM = 25.0
F_NUMBER = 2.0
PIXEL_SIZE = 3.45e-6
F_M = F_MM * 1e-3
PUPIL_DIAM = F_M / F_NUMBER
BATCH = 128
NCORES = 8
BPC = BATCH // NCORES          # batch per core (16)
NIMG = BPC * 2 * 3             # images per core (96)
G = 8                          # images per group
NGRP = NIMG // G               # 12 groups per core


def _noll_to_nm(j):
    n = 0
    k = j - 1
    while k > n:
        n += 1
        k -= n
    m = (-1) ** j * ((n % 2) + 2 * ((k + ((n + 1) % 2)) // 2))
    return n, m


def _zernike(n, m, r, theta):
    am = abs(m)
    R = np.zeros_like(r)
    for s in range((n - am) // 2 + 1):
        c = ((-1) ** s * factorial(n - s)
             / (factorial(s) * factorial((n + am) // 2 - s)
                * factorial((n - am) // 2 - s)))
        R = R + c * r ** (n - 2 * s)
    norm = np.sqrt(n + 1) if m == 0 else np.sqrt(2 * (n + 1))
    ang = np.cos(am * theta) if m >= 0 else np.sin(am * theta)
    return np.where(r <= 1.0, norm * R * ang, 0.0)


def _host_consts(wavelengths):
    """Input-independent structural constants (DFT/sampling matrices)."""
    N = GRID
    # shifted DFT: field = Ft X Ft, Ft[a,b] = F[(a+128)%256,(b+128)%256]
    idx = (np.arange(N) + N // 2) % N
    jk = np.outer(idx, idx).astype(np.float64)
    ang = -2.0 * np.pi * jk / N
    Fr_full = np.cos(ang)
    Fi_full = np.sin(ang)

    csel = np.zeros((3, 64), np.int64)      # tap rows per channel
    wt0 = np.zeros((3, 32), np.float32)     # tap weights
    wt1 = np.zeros((3, 32), np.float32)
    for i in range(3):
        lam = float(wavelengths[i])
        zoom = PIXEL_SIZE * FOV * PUPIL_DIAM / (lam * F_M * GRID)
        g1 = (np.arange(FOV, dtype=np.float32) / np.float32(FOV - 1)
              * np.float32(2.0 * zoom) - np.float32(zoom))
        x = ((g1 + 1.0) * GRID - 1.0) * 0.5
        x0 = np.floor(x)
        tx = (x - x0).astype(np.float32)
        x0 = x0.astype(np.int64)
        csel[i, 0:32] = x0
        csel[i, 32:64] = x0 + 1
        wt0[i] = 1.0 - tx
        wt1[i] = tx
    return Fr_full, Fi_full, csel, wt0, wt1


def build_nc():
    import concourse.bass as bass
    import concourse.bacc as bacc
    import concourse.mybir as mybir
    from concourse.tile import TileContext

    f32 = mybir.dt.float32
    bf16 = mybir.dt.bfloat16
    AF = mybir.ActivationFunctionType
    OP = mybir.AluOpType
    TWO_PI = float(2.0 * np.pi)

    nc = bacc.Bacc("TRN2", target_bir_lowering=False)
    # device inputs (per core)
    # Q^T tiles: per (f,i) p6: [QrT(ktmt 4) | QiT(4) | QinT(4)] each [128,128]
    qtd = nc.declare_dram_parameter("qt", [128, 6 * 12 * 128], bf16,
                                    isOutput=False)
    # X-build consts per channel i: [A_t0 | flip(A_t1)] with A = [FcrT|FciT],
    # B = [-FciT|FcrT].  Tile-1 halves are partition-reversed so tile-0 g
    # scalars apply (w(v) is symmetric); Q^T tiles are flipped to match.
    abd = nc.declare_dram_parameter("ab", [128, 3 * 256], bf16,
                                    isOutput=False)
    bbd = nc.declare_dram_parameter("bb", [128, 3 * 256], bf16,
                                    isOutput=False)
    wvd = nc.declare_dram_parameter("wv", [128, 1], f32, isOutput=False)
    erowd = nc.declare_dram_parameter("erow", [1, NIMG], f32, isOutput=False)
    wtd = nc.declare_dram_parameter("wt", [64, 3 * 32], bf16, isOutput=False)
    w0rd = nc.declare_dram_parameter("w0r", [64, 3 * G * 32], bf16,
                                     isOutput=False)
    w1rd = nc.declare_dram_parameter("w1r", [64, 3 * G * 32], bf16,
                                     isOutput=False)
    onesd = nc.declare_dram_parameter("ones32", [32, 1], f32, isOutput=False)
    outd = nc.declare_dram_parameter("out", [NIMG, 32, 32], f32, isOutput=True)

    with TileContext(nc) as tc:
        with (
            tc.tile_pool(name="const", bufs=1) as cpool,
            tc.tile_pool(name="g", bufs=1) as gpool,
            tc.tile_pool(name="xt", bufs=4) as xpool,
            tc.tile_pool(name="xs", bufs=2) as spool,
            tc.tile_pool(name="vsb", bufs=2) as vpool,
            tc.tile_pool(name="pp", bufs=2) as ppool,
            tc.tile_pool(name="fin", bufs=1) as opool,
            tc.tile_pool(name="psv", bufs=1, space="PSUM") as psv,
            tc.tile_pool(name="psb", bufs=2, space="PSUM") as psb,
            tc.tile_pool(name="ps3", bufs=1, space="PSUM") as ps3,
        ):
            # ---- load constants ----
            qt = cpool.tile([128, 6 * 12 * 128], bf16, tag="qt")
            nc.sync.dma_start(qt[:], qtd[:])
            ab = cpool.tile([128, 3 * 256], bf16, tag="ab")
            bb = cpool.tile([128, 3 * 256], bf16, tag="bb")
            nc.sync.dma_start(ab[:], abd[:])
            nc.sync.dma_start(bb[:], bbd[:])
            wv0 = cpool.tile([128, 1], f32, tag="wv0")
            erow0 = cpool.tile([1, NIMG], f32, tag="erow0")
            nc.sync.dma_start(wv0[:], wvd[:])
            nc.sync.dma_start(erow0[:], erowd[:])
            wt = cpool.tile([64, 3 * 32], bf16, tag="wt")
            w0r = cpool.tile([64, 3 * G * 32], bf16, tag="w0r")
            w1r = cpool.tile([64, 3 * G * 32], bf16, tag="w1r")
            nc.sync.dma_start(wt[:], wtd[:])
            nc.sync.dma_start(w0r[:], w0rd[:])
            nc.sync.dma_start(w1r[:], w1rd[:])
            ones0 = cpool.tile([32, 1], f32, tag="ones0")
            nc.sync.dma_start(ones0[:], onesd[:])
            # matmul lhsT wants a single producer sem: copy via one engine
            ones32 = cpool.tile([32, 1], f32, tag="ones32")
            nc.vector.tensor_copy(ones32[:], ones0[:])
            # erow broadcast to all partitions (for outer product via DVE)
            erow_b = gpool.tile([128, NIMG], f32, tag="erow_b")
            nc.gpsimd.partition_broadcast(erow_b[:], erow0[:])

            # ---- batched g vectors (tile-0 layout only; tile 1 reuses them
            # via the partition-reversed consts): gcos/gsin [128, NIMG]
            gcos = gpool.tile([128, NIMG], f32, tag="gcos")
            gsin = gpool.tile([128, NIMG], f32, tag="gsin")
            pg = gpool.tile([128, NIMG], f32, tag="pg")
            nc.vector.tensor_scalar_mul(pg[:], erow_b[:], wv0[:, 0:1])
            gm = gpool.tile([128, NIMG], f32, tag="gm")
            gmc = gpool.tile([128, NIMG], f32, tag="gmc")
            ua = gpool.tile([128, NIMG], f32, tag="ua")
            ub = gpool.tile([128, NIMG], f32, tag="ub")
            nc.vector.tensor_scalar_add(ua[:], pg[:], 256.0)
            nc.vector.tensor_scalar_add(ub[:], pg[:], 256.25)
            ui = gpool.tile([128, NIMG], mybir.dt.int32, tag="ui")
            uf = gpool.tile([128, NIMG], f32, tag="uf")
            nc.vector.tensor_copy(ui[:], ua[:])
            nc.vector.tensor_copy(uf[:], ui[:])
            nc.vector.tensor_sub(gm[:], ua[:], uf[:])
            nc.vector.tensor_copy(ui[:], ub[:])
            nc.vector.tensor_copy(uf[:], ui[:])
            nc.vector.tensor_sub(gmc[:], ub[:], uf[:])
            nc.scalar.activation(gsin[:], gm[:], AF.Sin, scale=TWO_PI)
            nc.scalar.activation(gcos[:], gmc[:], AF.Sin, scale=TWO_PI)

            o_all = opool.tile([32, NIMG * 32], f32, tag="o_all")
            o3 = o_all[:].rearrange("q (j p) -> q j p", p=32)

            # ---- per-group stages; software-pipelined emission so the PE
            # stream never waits on the elementwise chain of the same group.
            ctx = {}

            def group_params(grp):
                f = grp // 6
                i = (grp // 2) % 3
                h = grp % 2
                # image j indices: j = f*48 + b*3 + i, b = h*8 + m
                j0 = f * 48 + (h * G) * 3 + i
                return f, i, j0

            def build_x(grp):
                # xt [128, G*256]; per image m cols =
                #   [kt0: Xr(64)|Xi(64) | kt1(rev-partition): Xr|Xi]
                f, i, j0 = group_params(grp)
                xt = xpool.tile([128, G * 256], bf16, tag="xt")
                csl = slice(i * 256, (i + 1) * 256)
                for m in range(G):
                    j = j0 + 3 * m
                    gc = gcos[:, j: j + 1]
                    gs = gsin[:, j: j + 1]
                    msl = slice(m * 256, (m + 1) * 256)
                    t1 = spool.tile([128, 256], bf16, tag="t1")
                    t2 = spool.tile([128, 256], bf16, tag="t2")
                    nc.vector.tensor_scalar_mul(t1[:], ab[:, csl], gc)
                    nc.vector.tensor_scalar_mul(t2[:], bb[:, csl], gs)
                    nc.vector.tensor_add(xt[:, msl], t1[:], t2[:])
                ctx[grp] = {"xt": xt}

            def stage_a(grp):
                # V[part][mt] [128, G*64] f32 psum
                # Vr = Qr Xr + Qin Xi ; Vi = Qi Xr + Qr Xi
                f, i, j0 = group_params(grp)
                p6 = f * 3 + i
                xt = ctx[grp]["xt"]
                xt3 = xt[:].rearrange("p (m c) -> p m c", c=256)
                xr = [xt3[:, :, kt * 128: kt * 128 + 64] for kt in range(2)]
                xi = [xt3[:, :, kt * 128 + 64: kt * 128 + 128]
                      for kt in range(2)]

                def qtile(var, kt, mt):
                    # var: 0=QrT 1=QiT 2=QinT ; tile [128,128]
                    base = p6 * 12 * 128 + (var * 4 + kt * 2 + mt) * 128
                    return qt[:, base: base + 128]

                # Vi1 aliases Vr0's bank (vA): its matmuls WAR-wait the
                # early Vr0 copy, freeing a PSUM bank for pmb double-buffer.
                tags = ["vA", "vB", "vC", "vA"]
                vps = []
                for part in range(2):  # 0 = Vr, 1 = Vi
                    for mt in range(2):
                        vp = psv.tile([128, G * 64], f32,
                                      tag=tags[part * 2 + mt])
                        vps.append(vp)
                        for kt in range(2):
                            v1 = 0 if part == 0 else 1
                            v2 = 2 if part == 0 else 0
                            nc.tensor.matmul(vp[:], qtile(v1, kt, mt), xr[kt],
                                             start=(kt == 0), stop=False)
                            nc.tensor.matmul(vp[:], qtile(v2, kt, mt), xi[kt],
                                             start=False, stop=(kt == 1))
                ctx[grp]["vps"] = vps

            def vcopy(grp):
                # V PSUM -> SBUF bf16 on Act (Vr0 first: frees the vA bank)
                # vsb[mt] [128, G*128]: per image m cols = [Vr_m(64)|Vi_m(64)]
                vps = ctx[grp]["vps"]
                vsbs = []
                v3s = []
                for mt in range(2):
                    vsb = vpool.tile([128, G * 128], bf16, tag=f"vsb{mt}")
                    vsbs.append(vsb)
                    v3s.append(vsb[:].rearrange("p (m c) -> p m c", c=128))
                nc.scalar.copy(v3s[0][:, :, 0:64], vps[0][:])     # Vr0
                nc.scalar.copy(v3s[1][:, :, 0:64], vps[1][:])     # Vr1
                nc.scalar.copy(v3s[0][:, :, 64:128], vps[2][:])   # Vi0
                nc.scalar.copy(v3s[1][:, :, 64:128], vps[3][:])   # Vi1
                ctx[grp]["vsbs"] = vsbs

            def stage_b(grp):
                # per image pmB [128,128] of 4 products
                # lhsT = [Vr_mt m | Vi_mt m] (contiguous), rhs = xt kt-slice
                xt = ctx[grp]["xt"]
                vsbs = ctx[grp]["vsbs"]
                pmb = psb.tile([128, G * 128], f32, tag="pmb")
                for m in range(G):
                    for mt in range(2):
                        nc.tensor.matmul(
                            pmb[:, m * 128:(m + 1) * 128],
                            vsbs[mt][:, m * 128:(m + 1) * 128],
                            xt[:, m * 256 + mt * 128: m * 256 + mt * 128 + 128],
                            start=(mt == 0), stop=(mt == 1))
                ctx[grp]["pmb"] = pmb

            def chain_pre(grp):
                # bottom half (Vi-products) -> SBUF bf16 at partition 0.
                # Emitted FIRST in the iteration so the DVE d-ops (which sit
                # after the X build in the DVE stream) never stall on Act.
                pmb = ctx[grp]["pmb"]
                pmbot = ppool.tile([64, G * 128], bf16, tag="pmbot")
                nc.scalar.copy(pmbot[:], pmb[64:128, :])
                ctx[grp]["pmbot"] = pmbot

            def chain(grp):
                # pmb -> psf -> blended a1 (Act / DVE / Pool)
                f, i, j0 = group_params(grp)
                pmb = ctx[grp]["pmb"]
                pmb3 = pmb[:].rearrange("p (m x c) -> p m x c", m=G, x=2)
                pmbot = ctx[grp]["pmbot"]
                pmt3 = pmbot[:].rearrange("p (m x c) -> p m x c", m=G, x=2)
                # d1 = P11 - P22, d2 = P12 + P21  [64, G*64] bf16 (DVE)
                d1 = ppool.tile([64, G * 64], bf16, tag="d1")
                d2 = ppool.tile([64, G * 64], bf16, tag="d2")
                nc.vector.scalar_tensor_tensor(
                    d1[:], pmb3[0:64, :, 0, :], 1.0, pmt3[:, :, 1, :],
                    op0=OP.mult, op1=OP.subtract)
                nc.vector.scalar_tensor_tensor(
                    d2[:], pmb3[0:64, :, 1, :], 1.0, pmt3[:, :, 0, :],
                    op0=OP.mult, op1=OP.add)
                # squares (Act + Pool) + add (Pool): psf^T = d1^2 + d2^2
                sq1 = ppool.tile([64, G * 64], bf16, tag="sq1")
                sq2 = ppool.tile([64, G * 64], bf16, tag="sq2")
                nc.scalar.activation(sq1[:], d1[:], AF.Square)
                nc.gpsimd.tensor_mul(sq2[:], d2[:], d2[:])
                psf = ppool.tile([64, G * 64], bf16, tag="psf")
                nc.gpsimd.tensor_add(psf[:], sq1[:], sq2[:])
                # y-side 2-tap blend -> a1 [64, G*32]
                # y-taps stored [even-block(32) | odd-block(32)] per image
                psf3 = psf[:].rearrange("p (m two a) -> p m two a",
                                        m=G, two=2)
                wsl = slice(i * G * 32, (i + 1) * G * 32)
                w0v = w0r[:, wsl].rearrange("p (m a) -> p m a", m=G)
                w1v = w1r[:, wsl].rearrange("p (m a) -> p m a", m=G)
                ea = ppool.tile([64, G * 32], bf16, tag="ea")
                eb = ppool.tile([64, G * 32], bf16, tag="eb")
                a1 = ppool.tile([64, G * 32], bf16, tag="a1")
                ea3 = ea[:].rearrange("p (m a) -> p m a", m=G)
                eb3 = eb[:].rearrange("p (m a) -> p m a", m=G)
                nc.gpsimd.tensor_mul(ea3, psf3[:, :, 0, :], w0v)
                nc.gpsimd.tensor_mul(eb3, psf3[:, :, 1, :], w1v)
                nc.gpsimd.tensor_add(a1[:], ea[:], eb[:])
                ctx[grp]["a1"] = a1

            def stage3(grp):
                # x-side via matmul: [32(q), G*32(p)] + scatter into o_all
                f, i, j0 = group_params(grp)
                a1 = ctx[grp]["a1"]
                pm3 = ps3.tile([32, G * 32], f32, tag="pm3")
                nc.tensor.matmul(pm3[:], wt[:, i * 32:(i + 1) * 32], a1[:],
                                 start=True, stop=True)
                nc.scalar.copy(o3[:, j0: j0 + 3 * (G - 1) + 1: 3, :],
                               pm3[:].rearrange("q (m p) -> q m p", m=G))
                del ctx[grp]

            # Software pipeline, X built 2 groups ahead, elementwise chain
            # lagging 2 and stage3 lagging 3 so no engine waits cross-group.
            build_x(0)
            build_x(1)
            for it in range(NGRP + 3):
                if 0 <= it - 2 < NGRP:
                    chain_pre(it - 2)
                if it + 2 < NGRP:
                    build_x(it + 2)
                if it < NGRP:
                    stage_a(it)
                    vcopy(it)
                if 0 <= it - 1 < NGRP:
                    stage_b(it - 1)
                if 0 <= it - 2 < NGRP:
                    chain(it - 2)
                if 0 <= it - 3 < NGRP:
                    stage3(it - 3)

            # ---- batched finalize: sums, normalize, block-transpose, out
            csum = opool.tile([32, NIMG], f32, tag="csum")
            nc.vector.tensor_reduce(csum[:], o3, op=OP.add,
                                    axis=mybir.AxisListType.X)
            pcst = ps3.tile([32, G * 32], f32, tag="pm3")
            pcs = pcst[0:1, 0:NIMG]
            nc.tensor.matmul(pcs, ones32[:], csum[:], start=True, stop=True)
            rec = opool.tile([1, NIMG], f32, tag="rec")
            nc.vector.tensor_scalar_add(rec[:], pcs, 1e-8)
            nc.vector.reciprocal(rec[:], rec[:])
            recb = opool.tile([32, NIMG], f32, tag="recb")
            nc.gpsimd.partition_broadcast(recb[:], rec[:])
            t_all = opool.tile([32, NIMG * 32], f32, tag="t_all")
            nc.vector.transpose(t_all[:], o_all[:])
            for j in range(NIMG):
                jsl = slice(j * 32, (j + 1) * 32)
                nc.vector.tensor_scalar_mul(t_all[:, jsl], t_all[:, jsl],
                                            recb[:, j:j + 1])
            nc.sync.dma_start(outd[:].rearrange("j p q -> p j q"),
                              t_all[:].rearrange("p (j q) -> p j q", q=32))
    nc.compile()
    return nc


_CACHE = {}


def _get_nc():
    if "nc" not in _CACHE:
        _CACHE["nc"] = build_nc()
    return _CACHE["nc"]


def kernel(d_obj, current_focus_dist_0, current_focus_dist_90,
           zernike_0, zernike_90, zernike_basis, aperture, wavelengths):
    from concourse.bass_utils import run_bass_kernel_spmd
    import ml_dtypes
    bf = ml_dtypes.bfloat16

    d_obj = np.asarray(d_obj, np.float32)
    zernike_0 = np.asarray(zernike_0, np.float32)
    zernike_90 = np.asarray(zernike_90, np.float32)
    basis = np.asarray(zernike_basis, np.float32)
    aperture = np.asarray(aperture, np.float32)
    lam = np.asarray(wavelengths, np.float32)
    f0 = float(current_focus_dist_0)
    f90 = float(current_focus_dist_90)

    Fr_full, Fi_full, csel, wt0, wt1 = _host_consts(lam)

    # Q planes -> transposed tiles [QrT | QiT | QinT] per (f,i)
    O = np.tensordot(np.stack([zernike_0, zernike_90]),
                     basis.reshape(NZ, -1), axes=[[1], [0]])  # [2, 65536]
    O = O.reshape(2, GRID, GRID).astype(np.float64)
    qt = np.empty((128, 6 * 12 * 128), bf)
    for f in range(2):
        for i in range(3):
            ph = 2.0 * np.pi * O[f] / float(lam[i])
            Qr = (aperture * np.cos(ph))
            Qi = (aperture * np.sin(ph))
            p6 = f * 3 + i
            for var, Qm in enumerate((Qr, Qi, -Qi)):
                QT = Qm.T.astype(bf)        # [v, v']
                for kt in range(2):
                    for mt in range(2):
                        idx = p6 * 12 + var * 4 + kt * 2 + mt
                        T = QT[kt * 128:(kt + 1) * 128,
                               mt * 128:(mt + 1) * 128]
                        # tile-1 halves of X / V are partition-reversed
                        if kt == 1:
                            T = T[::-1, :]
                        if mt == 1:
                            T = T[:, ::-1]
                        qt[:, idx * 128:(idx + 1) * 128] = T

    # X-build consts: per channel i: [A_t0 | flipud(A_t1)], A = [FcrT|FciT],
    # B = [-FciT|FcrT] (tile-1 partition-reversed to reuse tile-0 g scalars)
    ab = np.empty((128, 3 * 256), bf)
    bb = np.empty((128, 3 * 256), bf)
    for i in range(3):
        FcrT = Fr_full[:, csel[i]].astype(np.float32)   # [256, 64]
        FciT = Fi_full[:, csel[i]].astype(np.float32)
        A = np.concatenate([FcrT, FciT], 1)             # [256, 128]
        B = np.concatenate([-FciT, FcrT], 1)
        sl = slice(i * 256, (i + 1) * 256)
        ab[:, sl] = np.concatenate([A[0:128], A[128:256][::-1]], 1).astype(bf)
        bb[:, sl] = np.concatenate([B[0:128], B[128:256][::-1]], 1).astype(bf)

    # wv col (tile 0 only): 2*v^2 - 0.5 on the [-1,1] grid, [128, 1]
    lin = np.linspace(-1.0, 1.0, GRID)
    wv = np.ascontiguousarray(
        (2.0 * lin * lin - 0.5).astype(np.float32)[0:128, None])

    # W^T (x-blend) and y-tap weight mats (row-replicated, per-image tiled)
    wt = np.zeros((64, 3 * 32), np.float32)
    w0r = np.zeros((64, 3 * G * 32), np.float32)
    w1r = np.zeros((64, 3 * G * 32), np.float32)
    for i in range(3):
        W = np.zeros((32, 64), np.float32)
        for p in range(32):
            W[p, p] = wt0[i, p]
            W[p, 32 + p] = wt1[i, p]
        wt[:, i * 32:(i + 1) * 32] = W.T
        w0r[:, i * G * 32:(i + 1) * G * 32] = \
            np.tile(wt0[i][None, :], (64, G))
        w1r[:, i * G * 32:(i + 1) * G * 32] = \
            np.tile(wt1[i][None, :], (64, G))

    # defocus scalars -> e_j = delta * sqrt(3) / lam
    delta = np.stack([
        F_M ** 2 / (8.0 * F_NUMBER ** 2) * (1.0 / f0 - 1.0 / (d_obj + 1e-8)),
        F_M ** 2 / (8.0 * F_NUMBER ** 2) * (1.0 / f90 - 1.0 / (d_obj + 1e-8)),
    ])  # [2, 128]

    nc = _get_nc()
    in_maps = []
    for c in range(NCORES):
        erow = np.empty((1, NIMG), np.float32)
        for j in range(NIMG):
            f = j // (BPC * 3)
            b = (j // 3) % BPC
            i = j % 3
            erow[0, j] = delta[f, c * BPC + b] * np.sqrt(3.0) / float(lam[i])
        in_maps.append({
            "qt": qt, "ab": ab, "bb": bb,
            "wv": wv, "erow": erow,
            "wt": wt.astype(bf), "w0r": w0r.astype(bf), "w1r": w1r.astype(bf),
            "ones32": np.ones((32, 1), np.float32),
        })
    trace = bool(_CACHE.get("trace"))
    res = run_bass_kernel_spmd(nc, in_maps, list(range(NCORES)), trace=trace)
    _CACHE["last_res"] = res
    outs = res.results
    psf0 = np.empty((BATCH, 3, FOV, FOV), np.float32)
    psf90 = np.empty((BATCH, 3, FOV, FOV), np.float32)
    for c in range(NCORES):
        o = np.asarray(outs[c]["out"]).reshape(2, BPC, 3, FOV, FOV)
        psf0[c * BPC:(c + 1) * BPC] = o[0]
        psf90[c * BPC:(c + 1) * BPC] = o[1]
    return psf0, psf90
